# revision 1
# baseline (speedup 1.0000x reference)
"""MixMOE forward on 8 Trainium2 NeuronCores.

Strategy (expert-parallel, sparse dispatch):
  - Host computes the tiny NaiveGate routing (logits -> top-2 -> softmax),
    in float64 (bitwise-stable ordering; verified the #2/#3 logit gap is
    >> fp32 noise so routing matches the fp32 reference exactly).
  - Tokens are gathered per expert (the "all-to-all dispatch"), transposed
    to [D, C] blocks, and shipped to the core that owns the expert
    (2 experts per core, 8 cores).
  - Each core runs the expert FFN as weights-stationary / tokens-moving
    matmuls in float32r (full-rate fp32 at ~1.4e-4 matmul rel-err):
        hT = gelu_tanh(W1_e^T @ xT + b1)   [H, C]
        yT = W2_e^T @ hT                   [D, C]
  - Host applies gate weights + b2 during the scatter-add combine
    (the "all-to-all return").
Compute is 8x less than the dense-equivalent reference (top-2 of 16).
"""

import sys

sys.path.insert(0, "/opt/trn_rl_repo")

import numpy as np

T, D, H, E, TOP_K, NCORES = 2048, 1024, 2048, 16, 2, 8
EPC = E // NCORES  # experts per core
DT8 = D // 128  # 8 d-tiles
HT16 = H // 128  # 16 h-tiles

_CACHE: dict = {}


def _build(C: int, dt_name: str):
    """Build + finalize the per-core Bass program (SPMD across 8 cores)."""
    import concourse.bacc as bacc
    import concourse.mybir as mybir
    from concourse.tile import TileContext

    DT = getattr(mybir.dt, dt_name)
    f32 = mybir.dt.float32

    nc = bacc.Bacc("TRN2", target_bir_lowering=False)
    xt = nc.dram_tensor("xt", [EPC, D, C], DT, kind="ExternalInput")
    w1 = nc.dram_tensor("w1", [EPC, D, H], DT, kind="ExternalInput")
    w2 = nc.dram_tensor("w2", [EPC, H, D], DT, kind="ExternalInput")
    b1 = nc.dram_tensor("b1", [EPC, H], f32, kind="ExternalInput")
    yt = nc.dram_tensor("yt", [EPC, D, C], f32, kind="ExternalOutput")

    gelu = mybir.ActivationFunctionType.Gelu_apprx_tanh

    with TileContext(nc) as tc:
        with (
            tc.tile_pool(name="xpool", bufs=2) as xpool,
            tc.tile_pool(name="w1pool", bufs=4) as w1pool,
            tc.tile_pool(name="w2pool", bufs=4) as w2pool,
            tc.tile_pool(name="hpool", bufs=2) as hpool,
            tc.tile_pool(name="opool", bufs=3) as opool,
            tc.tile_pool(name="cpool", bufs=1) as cpool,
            tc.tile_pool(name="ps1", bufs=3, space="PSUM") as ps1,
            tc.tile_pool(name="ps2", bufs=3, space="PSUM") as ps2,
        ):
            b1t = cpool.tile([128, EPC * HT16], f32)
            nc.sync.dma_start(
                out=b1t, in_=b1.rearrange("e (ht p) -> p (e ht)", p=128)
            )
            for e in range(EPC):
                xte = xpool.tile([128, DT8, C], DT, tag="xt")
                nc.sync.dma_start(
                    out=xte, in_=xt[e].rearrange("(dt p) c -> p dt c", p=128)
                )
                hsb = hpool.tile([128, HT16, C], DT, tag="ht")
                # --- GEMM1 + gelu: hT[ht] = gelu(sum_dt W1[dt,ht].T @ xT[dt] + b1) ---
                for ht in range(HT16):
                    w1t = w1pool.tile([128, DT8, 128], DT, tag="w1")
                    nc.sync.dma_start(
                        out=w1t,
                        in_=w1[e, :, ht * 128 : (ht + 1) * 128].rearrange(
                            "(dt p) m -> p dt m", p=128
                        ),
                    )
                    acc = ps1.tile([128, C], f32, tag="ps1")
                    for dt in range(DT8):
                        nc.tensor.matmul(
                            acc[:],
                            w1t[:, dt, :],
                            xte[:, dt, :],
                            start=(dt == 0),
                            stop=(dt == DT8 - 1),
                        )
                    nc.scalar.activation(
                        hsb[:, ht, :],
                        acc[:],
                        gelu,
                        bias=b1t[:, e * HT16 + ht : e * HT16 + ht + 1],
                    )
                # --- GEMM2: yT[dt2] = sum_ht W2[ht,dt2].T @ hT[ht] ---
                for dt2 in range(DT8):
                    w2t = w2pool.tile([128, HT16, 128], DT, tag="w2")
                    nc.sync.dma_start(
                        out=w2t,
                        in_=w2[e, :, dt2 * 128 : (dt2 + 1) * 128].rearrange(
                            "(ht p) m -> p ht m", p=128
                        ),
                    )
                    acc2 = ps2.tile([128, C], f32, tag="ps2")
                    for ht in range(HT16):
                        nc.tensor.matmul(
                            acc2[:],
                            w2t[:, ht, :],
                            hsb[:, ht, :],
                            start=(ht == 0),
                            stop=(ht == HT16 - 1),
                        )
                    ot = opool.tile([128, C], f32, tag="ot")
                    nc.vector.tensor_copy(ot[:], acc2[:])
                    nc.sync.dma_start(
                        out=yt[e, dt2 * 128 : (dt2 + 1) * 128, :], in_=ot[:]
                    )
    nc.finalize()
    return nc


def _route(x: np.ndarray, gate_w: np.ndarray):
    """Host NaiveGate: fp64 logits -> stable top-2 -> softmax. Returns
    (top_idx [T,2] int, gate_score [T,2] f64)."""
    logits = x.astype(np.float64) @ gate_w.astype(np.float64)
    top_idx = np.argsort(-logits, axis=1, kind="stable")[:, :TOP_K]
    top_val = np.take_along_axis(logits, top_idx, axis=1)
    ex = np.exp(top_val - top_val.max(axis=1, keepdims=True))
    gate = ex / ex.sum(axis=1, keepdims=True)
    return top_idx, gate


def _run_device(nc, in_maps, trace=False, tmpdir=None):
    from concourse.bass_utils import run_bass_kernel_spmd

    return run_bass_kernel_spmd(
        nc, in_maps, core_ids=list(range(NCORES)), trace=trace, tmpdir=tmpdir
    )


def kernel(x, gate_w, W1, b1, W2, b2, _trace=False, _tmpdir=None):
    x = np.ascontiguousarray(np.asarray(x, dtype=np.float32))
    gate_w = np.asarray(gate_w, dtype=np.float32)
    W1 = np.asarray(W1, dtype=np.float32)
    b1 = np.asarray(b1, dtype=np.float32)
    W2 = np.asarray(W2, dtype=np.float32)
    b2 = np.asarray(b2, dtype=np.float32)

    top_idx, gate = _route(x, gate_w)

    # Token lists per expert
    idx_e = [np.where(top_idx == e)[0] for e in range(E)]
    gat_e = [gate[top_idx == e] for e in range(E)]
    cmax = max(len(i) for i in idx_e)
    C = max(256, -(-cmax // 32) * 32)  # >=256 keeps float32r at full rate

    dt_name = "float32r"
    key = (C, dt_name)
    if key not in _CACHE:
        _CACHE[key] = _build(C, dt_name)
    nc = _CACHE[key]

    in_maps = []
    for core in range(NCORES):
        xt = np.zeros((EPC, D, C), np.float32)
        for s in range(EPC):
            e = core * EPC + s
            ids = idx_e[e]
            xt[s, :, : len(ids)] = x[ids].T
        es = slice(core * EPC, (core + 1) * EPC)
        in_maps.append(
            {
                "xt": xt,
                "w1": np.ascontiguousarray(W1[es]),
                "w2": np.ascontiguousarray(W2[es]),
                "b1": np.ascontiguousarray(b1[es]),
            }
        )

    res = _run_device(nc, in_maps, trace=_trace, tmpdir=_tmpdir)

    out = np.zeros((T, D), np.float32)
    for e in range(E):
        core, s = divmod(e, EPC)
        ids = idx_e[e]
        if len(ids) == 0:
            continue
        y = res.results[core]["yt"][s][:, : len(ids)].T  # [C_e, D]
        out[ids] += (gat_e[e][:, None] * (y + b2[e][None, :])).astype(np.float32)

    if _trace:
        return out, res
    return out


# revision 2
# speedup vs baseline: 1.8401x; 1.8401x over previous
"""MixMOE forward on 8 Trainium2 NeuronCores.

Strategy (expert-parallel, sparse dispatch):
  - Host computes the tiny NaiveGate routing (logits -> top-2 -> softmax)
    in float64 (bitwise-stable ordering; the #2/#3 logit gap is >> fp32
    noise for this problem size, so routing matches the fp32 reference).
  - Tokens are gathered per expert (the "all-to-all dispatch"), transposed
    to [D, C] blocks, and shipped to the core that owns the expert
    (2 experts per core, 8 cores). Weights are host-pre-tiled so every
    device DMA is fully contiguous.
  - Each core runs its expert FFNs as weights-stationary / tokens-moving
    fp16 matmuls (fp32 PSUM accumulation, ~4e-4 end-to-end rel-err):
        hT = gelu_tanh(W1_e^T @ xT + b1)   [H, C]
        yT = W2_e^T @ hT                   [D, C]
  - Host applies gate weights + b2 during the scatter-add combine
    (the "all-to-all return").
Compute is 8x less than the dense-equivalent reference (top-2 of 16).
"""

import sys

sys.path.insert(0, "/opt/trn_rl_repo")

import numpy as np

T, D, H, E, TOP_K, NCORES = 2048, 1024, 2048, 16, 2, 8
EPC = E // NCORES  # experts per core
DT8 = D // 128  # 8 d-tiles
HT16 = H // 128  # 16 h-tiles

_CACHE: dict = {}


def _build(C: int, dt_name: str):
    """Build + finalize the per-core Bass program (SPMD across 8 cores)."""
    import concourse.bacc as bacc
    import concourse.mybir as mybir
    from concourse.tile import TileContext

    DT = getattr(mybir.dt, dt_name)
    f32 = mybir.dt.float32

    nc = bacc.Bacc("TRN2", target_bir_lowering=False)
    # Host-pre-tiled layouts: every DMA below is contiguous in HBM.
    xt = nc.dram_tensor("xt", [EPC, 128, DT8, C], DT, kind="ExternalInput")
    w1 = nc.dram_tensor("w1", [EPC, HT16, 128, DT8, 128], DT, kind="ExternalInput")
    w2 = nc.dram_tensor("w2", [EPC, DT8, 128, HT16, 128], DT, kind="ExternalInput")
    b1 = nc.dram_tensor("b1", [EPC, H], f32, kind="ExternalInput")
    yt = nc.dram_tensor("yt", [EPC, DT8, 128, C], f32, kind="ExternalOutput")

    gelu = mybir.ActivationFunctionType.Gelu_apprx_tanh

    with TileContext(nc) as tc:
        with (
            tc.tile_pool(name="xpool", bufs=2) as xpool,
            tc.tile_pool(name="w1pool", bufs=10) as w1pool,
            tc.tile_pool(name="w2pool", bufs=10) as w2pool,
            tc.tile_pool(name="hpool", bufs=2) as hpool,
            tc.tile_pool(name="opool", bufs=3) as opool,
            tc.tile_pool(name="cpool", bufs=1) as cpool,
            tc.tile_pool(name="ps1", bufs=4, space="PSUM") as ps1,
            tc.tile_pool(name="ps2", bufs=4, space="PSUM") as ps2,
        ):
            b1t = cpool.tile([128, EPC * HT16], f32)
            nc.sync.dma_start(
                out=b1t, in_=b1.rearrange("e (ht p) -> p (e ht)", p=128)
            )
            for e in range(EPC):
                xte = xpool.tile([128, DT8, C], DT, tag="xt")
                nc.sync.dma_start(out=xte, in_=xt[e])
                hsb = hpool.tile([128, HT16, C], DT, tag="ht")
                # --- GEMM1 + gelu: hT[ht] = gelu(sum_dt W1[ht,:,dt].T @ xT[dt] + b1) ---
                for ht in range(HT16):
                    w1t = w1pool.tile([128, DT8, 128], DT, tag="w1")
                    nc.sync.dma_start(out=w1t, in_=w1[e, ht])
                    acc = ps1.tile([128, C], f32, tag="ps1")
                    for dt in range(DT8):
                        nc.tensor.matmul(
                            acc[:],
                            w1t[:, dt, :],
                            xte[:, dt, :],
                            start=(dt == 0),
                            stop=(dt == DT8 - 1),
                        )
                    nc.scalar.activation(
                        hsb[:, ht, :],
                        acc[:],
                        gelu,
                        bias=b1t[:, e * HT16 + ht : e * HT16 + ht + 1],
                    )
                # --- GEMM2: yT[dt2] = sum_ht W2[dt2,:,ht].T @ hT[ht] ---
                for dt2 in range(DT8):
                    w2t = w2pool.tile([128, HT16, 128], DT, tag="w2")
                    nc.sync.dma_start(out=w2t, in_=w2[e, dt2])
                    acc2 = ps2.tile([128, C], f32, tag="ps2")
                    for ht in range(HT16):
                        nc.tensor.matmul(
                            acc2[:],
                            w2t[:, ht, :],
                            hsb[:, ht, :],
                            start=(ht == 0),
                            stop=(ht == HT16 - 1),
                        )
                    ot = opool.tile([128, C], f32, tag="ot")
                    nc.vector.tensor_copy(ot[:], acc2[:])
                    nc.sync.dma_start(out=yt[e, dt2], in_=ot[:])
    nc.finalize()
    return nc


def _route(x: np.ndarray, gate_w: np.ndarray):
    """Host NaiveGate: fp64 logits -> stable top-2 -> softmax. Returns
    (top_idx [T,2] int, gate_score [T,2] f64)."""
    logits = x.astype(np.float64) @ gate_w.astype(np.float64)
    top_idx = np.argsort(-logits, axis=1, kind="stable")[:, :TOP_K]
    top_val = np.take_along_axis(logits, top_idx, axis=1)
    ex = np.exp(top_val - top_val.max(axis=1, keepdims=True))
    gate = ex / ex.sum(axis=1, keepdims=True)
    return top_idx, gate


def _run_device(nc, in_maps, trace=False, tmpdir=None):
    from concourse.bass_utils import run_bass_kernel_spmd

    return run_bass_kernel_spmd(
        nc, in_maps, core_ids=list(range(NCORES)), trace=trace, tmpdir=tmpdir
    )


DT_NAME = "float16"
_NPDT = {"float16": np.float16, "float32r": np.float32, "bfloat16": None}


def _npdt(dt_name):
    if dt_name == "bfloat16":
        import ml_dtypes

        return ml_dtypes.bfloat16
    return _NPDT[dt_name]


def kernel(x, gate_w, W1, b1, W2, b2, _trace=False, _tmpdir=None):
    x = np.ascontiguousarray(np.asarray(x, dtype=np.float32))
    gate_w = np.asarray(gate_w, dtype=np.float32)
    W1 = np.asarray(W1, dtype=np.float32)
    b1 = np.asarray(b1, dtype=np.float32)
    W2 = np.asarray(W2, dtype=np.float32)
    b2 = np.asarray(b2, dtype=np.float32)

    top_idx, gate = _route(x, gate_w)

    idx_e = [np.where(top_idx == e)[0] for e in range(E)]
    gat_e = [gate[top_idx == e] for e in range(E)]
    cmax = max(len(i) for i in idx_e)
    C = max(256, -(-cmax // 32) * 32)

    npdt = _npdt(DT_NAME)
    key = (C, DT_NAME)
    if key not in _CACHE:
        _CACHE[key] = _build(C, DT_NAME)
    nc = _CACHE[key]

    in_maps = []
    for core in range(NCORES):
        es = slice(core * EPC, (core + 1) * EPC)
        xt = np.zeros((EPC, 128, DT8, C), npdt)
        for s in range(EPC):
            ids = idx_e[core * EPC + s]
            # [C_e, D] -> [D, C_e] -> [dt, p, C_e] -> [p, dt, C_e]
            xg = x[ids].T.reshape(DT8, 128, len(ids)).transpose(1, 0, 2)
            xt[s, :, :, : len(ids)] = xg
        # W1[e,d,h] -> [e, ht, p, dt, m];  W2[e,h,d] -> [e, dt2, p, ht, m]
        w1p = np.ascontiguousarray(
            W1[es].reshape(EPC, DT8, 128, HT16, 128).transpose(0, 3, 2, 1, 4),
            dtype=npdt,
        )
        w2p = np.ascontiguousarray(
            W2[es].reshape(EPC, HT16, 128, DT8, 128).transpose(0, 3, 2, 1, 4),
            dtype=npdt,
        )
        in_maps.append(
            {
                "xt": xt,
                "w1": w1p,
                "w2": w2p,
                "b1": np.ascontiguousarray(b1[es]),
            }
        )

    res = _run_device(nc, in_maps, trace=_trace, tmpdir=_tmpdir)

    out = np.zeros((T, D), np.float32)
    for e in range(E):
        core, s = divmod(e, EPC)
        ids = idx_e[e]
        if len(ids) == 0:
            continue
        # yt: [dt2, 128, C] -> [D, C_e] -> [C_e, D]
        y = res.results[core]["yt"][s].reshape(D, C)[:, : len(ids)].T
        out[ids] += (gat_e[e][:, None] * (y + b2[e][None, :])).astype(np.float32)

    if _trace:
        return out, res
    return out
